# revision 6
# baseline (speedup 1.0000x reference)
"""Trainium2 Bass kernel for multi-head attention (B=2, L=S=4096, H=8, E=64).

  scores = einsum('blhe,bshe->bhls', q, k) * E**-0.5
  attn   = softmax(scores, axis=-1)
  out    = einsum('bhls,bshd->blhd', attn, v)

Sharding: B*H = 16 (batch, head) pairs -> 8 cores, 2 adjacent heads of one
batch per core. Each core runs dense attention for its 2 heads; no
cross-core communication.

v3: QK row-tile 2-pack. The PE's 128x128 array is 16 independent 32x32
subarrays; matmuls whose tiles occupy disjoint row groups run
CONCURRENTLY when their moving operands sit on disjoint partition ranges
at the same free-dim addresses (skill: 4x K=32 row tiles measured
3.07x). Our two heads are exactly that shape: kT rows 0:64 = head0 E,
64:128 = head1 E, and qT likewise. Issuing head0's and head1's QK chunk
back-to-back (tile_position (0,0) / (64,0), outputs to different PSUM
banks) streams both heads' scores in one 512-cycle pass - QK drops from
~109us to ~55us per core.

Structure: loop over the 8 l-tiles; each iteration computes BOTH heads.
  - 21 QK groups per l-tile: [A(2 cpairs), B(1 cpair)] x 10 + [A] where
    a cpair = (head0 chunk c, head1 chunk c). A-groups live in a 4-bank
    PSUM pool, B-groups in a 2-bank pool; the remaining 2 banks hold the
    two heads' PV accumulators [65, 512] (ones column in vx row 64
    accumulates the softmax denominator).
  - exp: A-groups on ACT (2048-elem ACTIVATEs, exp with scale folded),
    8 of 10 B-groups on DVE via Schraudolph fast-exp (int32(A*x+B)
    bitcast to f32, ~3-4% max weight error, washes out in the softmax
    average), 2 B-groups on ACT.
  - PV of l-tile i-1 interleaves into i's QK groups in 4 runs of 16
    (v-stationary, moving = attn tile [128 s, 512 l]).
  - phase A: staged 8-chunk DMA loads (k0,q0 first), fp32 PE transposes
    through pvo (first k+q batch) then pool scratch, DVE cast to bf16.
    Identity arrives by DMA (ExternalInput) instead of a ~6us gpsimd
    make_identity.
  - finalize: copy [65, 2, 512] PSUM to SBUF, DMA as o[h] = [E+1, L];
    softmax division + transpose to [L, H, E] on host.
"""

import numpy as np

P = 128
E = 64
NH = 2   # heads per core
L = 4096
S = 4096
LT = 512          # l-tile (moving dim of QK, free dim of PV psum)
NS = S // P       # 32 s-chunks
NLT = L // LT     # 8 l-tiles
SUP = 8           # chunks per batched load

# QK group schedule per l-tile: ('A', 2 cpairs) uses the 4-bank pool,
# ('B', 1 cpair) the 2-bank pool. 11 A + 10 B = 32 cpairs.
GROUPS = [('A', 2), ('B', 1)] * 10 + [('A', 2)]
assert sum(n for _, n in GROUPS) == NS
NG = len(GROUPS)
# which B-groups (by group index) run exp on the DVE instead of ACT
DVE_GROUPS = (1, 3, 5, 7, 11, 13, 15, 17)


def _build(num_devices=8):
    import concourse.mybir as mybir
    import concourse.tile as tile
    from concourse import bacc

    f32 = mybir.dt.float32
    bf16 = mybir.dt.bfloat16
    i32 = mybir.dt.int32
    Exp = mybir.ActivationFunctionType.Exp
    Mult = mybir.AluOpType.mult
    Add = mybir.AluOpType.add

    scale = float(E) ** -0.5
    # Schraudolph fast-exp constants (see docstring)
    SCHRA_A = float((1 << 23) * scale / np.log(2.0))
    SCHRA_B = float(127 * (1 << 23) - 361004 + 0.5)

    nc = bacc.Bacc(
        "TRN2", target_bir_lowering=False, debug=False, num_devices=num_devices
    )
    q = nc.dram_tensor("q", [L, NH, E], f32, kind="ExternalInput").ap()
    k = nc.dram_tensor("k", [S, NH, E], f32, kind="ExternalInput").ap()
    v = nc.dram_tensor("v", [S, NH, E], f32, kind="ExternalInput").ap()
    iden = nc.dram_tensor("iden", [P, P], f32, kind="ExternalInput").ap()
    o = nc.dram_tensor("o", [NH, E + 1, L], f32, kind="ExternalOutput").ap()

    with tile.TileContext(nc) as tc:
        with (
            tc.tile_pool(name="persist", bufs=1) as persist,
            tc.tile_pool(name="stage", bufs=8) as stage,
            tc.tile_pool(name="attn", bufs=2) as attn_pool,
            tc.tile_pool(name="outp", bufs=2) as outp,
            tc.tile_pool(name="sexp", bufs=2) as sexp,
            tc.tile_pool(name="qkA", bufs=1, space="PSUM") as qkA,
            tc.tile_pool(name="qkB", bufs=1, space="PSUM") as qkB,
            tc.tile_pool(name="pvo", bufs=1, space="PSUM") as pvo,
        ):
            ident = persist.tile([P, P], f32, name="ident")

            # persistent bf16 operands: rows 0:64 = head0 E, 64:128 = head1
            kT = persist.tile([P, NS * P], bf16, name="kT")
            qT = persist.tile([P, L], bf16, name="qT")
            # v chunks with a ones column (denominator accumulator)
            vx = persist.tile([P, NS, NH, E + 1], bf16, name="vx")

            # loads first (HWDGE spin-up ~3.5us): identity, then staged q/k
            # super-chunks with k0/q0 leading.
            nc.sync.dma_start(ident[:], iden[:, :])
            n_sup = NS // SUP
            ks, qs = [None] * n_sup, [None] * n_sup
            order = [(k, ks, 0), (q, qs, 0), (k, ks, 1), (k, ks, 2),
                     (k, ks, 3), (q, qs, 1), (q, qs, 2), (q, qs, 3)]
            for src, dst, b in order:
                st = stage.tile([P, SUP, NH * E], f32, name="st")
                nc.sync.dma_start(
                    st[:],
                    src[b * SUP * P : (b + 1) * SUP * P, :, :].rearrange(
                        "(j p) h e -> p j (h e)", p=P
                    ),
                )
                dst[b] = st

            # ---- phase A ----
            # ones column first, then converting v loads via gpsimd SWDGE.
            nc.gpsimd.memset(vx[:, :, :, E : E + 1], 1.0)
            for c in range(NS):
                nc.gpsimd.dma_start(
                    vx[:, c, :, 0:E], v[c * P : (c + 1) * P, :, :]
                )

            # PE prewarm through qkB (HAM gate): dummy transposes as soon
            # as the identity lands, before k0 arrives.
            warm_ps = qkB.tile([P, 2, LT], f32, name="ps")
            for i in range(8):
                nc.tensor.transpose(
                    warm_ps[:, i % 2, 0:P], ident[:], ident[:]
                )

            kw = [(c, ks[c // SUP][:, c % SUP, :]) for c in range(NS)]
            qw = [(c, qs[c // SUP][:, c % SUP, :]) for c in range(NS)]

            def emit_batch(pool, rows, batch):
                # 4-transpose micro-blocks, each drained by ONE [128, 512]
                # cast into the big kT tile.
                ps = pool.tile([P, rows, LT], f32, name="ps")
                for b0 in range(0, len(batch), 4):
                    blk = batch[b0 : b0 + 4]
                    for s, (c, src) in enumerate(blk, b0):
                        nc.tensor.transpose(
                            ps[:, s // 4, (s % 4) * P : (s % 4 + 1) * P],
                            src, ident,
                        )
                    c0 = blk[0][0]
                    nc.vector.tensor_copy(
                        kT[:, c0 * P : (c0 + len(blk)) * P],
                        ps[:, b0 // 4, : len(blk) * P],
                    )

            def emit_q_batch(bq, slot=None):
                # one l-tile's worth of q (4 chunks) through a pvo slot
                if slot is None:
                    slot = pvo.tile([P, NH, LT], f32, name="po")[:, 1, :]
                batch = qw[4 * bq : 4 * bq + 4]
                for s, (c, src) in enumerate(batch):
                    nc.tensor.transpose(
                        slot[:, s * P : (s + 1) * P], src, ident
                    )
                nc.vector.tensor_copy(
                    qT[:, bq * LT : (bq + 1) * LT], slot[:]
                )

            # first k batch (chunks 0-3) and first q batch share one pvo
            # tile (slot 0 / slot 1) so QK group 0 only waits on them.
            po0 = pvo.tile([P, NH, LT], f32, name="po")
            for s, (c, src) in enumerate(kw[0:4]):
                nc.tensor.transpose(
                    po0[:, 0, s * P : (s + 1) * P], src, ident
                )
            nc.vector.tensor_copy(kT[:, 0 : 4 * P], po0[:, 0, :])
            emit_q_batch(0, slot=po0[:, 1, :])

            emit_batch(qkA, 3, kw[4:16])
            emit_batch(qkB, 2, kw[16:24])
            emit_batch(qkB, 2, kw[24:32])

            # ---- main loop over l-tiles, both heads per iteration ----
            at_tiles = {}
            po_tiles = {}

            def emit_pv(i, run):
                # run r: head r//2, chunks (r%2)*16 .. +16
                h, cb = run // 2, (run % 2) * 16
                if run == 0:
                    po_tiles[i] = pvo.tile([P, NH, LT], f32, name="po")
                po = po_tiles[i]
                at = at_tiles[i]
                for c in range(cb, cb + 16):
                    nc.tensor.matmul(
                        po[0 : E + 1, h, :],
                        lhsT=vx[:, c, h, :],
                        rhs=at[:, c, h, :],
                        start=(c == 0),
                        stop=(c == NS - 1),
                    )

            def emit_finalize(i):
                po = po_tiles.pop(i)
                of = outp.tile([E + 1, NH, LT], f32, name="of")
                nc.vector.tensor_copy(of[:], po[0 : E + 1, :, :])
                for h in range(NH):
                    nc.sync.dma_start(
                        o[h, :, i * LT : (i + 1) * LT], of[:, h, :]
                    )

            last = NLT - 1
            for i in range(NLT):
                at = attn_pool.tile([P, NS, NH, LT], bf16, name="at")
                at_tiles[i] = at
                cp = 0  # chunk-pair index
                nq = 1  # next q batch (pair 0 only)
                npv = 0  # next PV run of i-1
                nspv = 0  # next PV run of i itself (last l-tile only)
                # last l-tile: run PV(i-1) early, finalize it mid-stream,
                # then overlap PV(i) with i's own later exp groups so the
                # end-of-kernel tail isn't a bare 13.7us PV run.
                pv_at = (2, 5, 8, 11) if i == last else (4, 9, 14, 19)
                for g, (typ, ncp) in enumerate(GROUPS):
                    if i == last and g == 13:
                        emit_finalize(i - 1)
                    if i == last and g in (15, 18):
                        # runs 0/2 (chunks 0-15 of each head) only depend
                        # on exp groups <= 10, all emitted by now
                        emit_pv(i, nspv)
                        nspv += 2
                    pool, banks = (qkA, 2) if typ == 'A' else (qkB, 1)
                    ps = pool.tile([P, banks, NH, LT], f32, name="ps")
                    for j in range(ncp):
                        c = cp + j
                        for h in range(NH):
                            h0 = E * h
                            nc.tensor.matmul(
                                ps[:, j, h, :],
                                lhsT=kT[h0 : h0 + E, c * P : (c + 1) * P],
                                rhs=qT[h0 : h0 + E, i * LT : (i + 1) * LT],
                                start=True,
                                stop=True,
                            )
                    if g in DVE_GROUPS:
                        # Schraudolph fast-exp on the DVE
                        ti = sexp.tile([P, NH, LT], i32, name="ti")
                        nc.vector.tensor_scalar(
                            ti[:], ps[:, 0, :, :],
                            SCHRA_A, SCHRA_B, Mult, Add,
                        )
                        nc.vector.tensor_copy(
                            at[:, cp, :, :], ti[:].bitcast(f32)
                        )
                    else:
                        nc.scalar.activation(
                            at[:, cp : cp + ncp, :, :], ps[:, :ncp, :, :],
                            Exp, scale=scale,
                        )
                    # interleave PV runs of l-tile i-1 / q batches (pair 0)
                    if i > 0 and g in pv_at:
                        emit_pv(i - 1, npv)
                        npv += 1
                    elif i == 0 and g % 3 == 2 and nq < 8:
                        emit_q_batch(nq)
                        nq += 1
                    cp += ncp
                if i == last:
                    emit_pv(i, 1)
                    emit_pv(i, 3)
                    emit_finalize(i)
                elif i > 0:
                    emit_finalize(i - 1)
                    at_tiles.pop(i - 1)

    nc.compile()
    return nc


_CACHE = {}


def _get_nc():
    if "nc" not in _CACHE:
        _CACHE["nc"] = _build()
    return _CACHE["nc"]


def kernel(q, k, v):
    from concourse.bass_utils import run_bass_kernel_spmd

    q = np.asarray(q)
    k = np.asarray(k)
    v = np.asarray(v)
    B, Lq, H, _E = q.shape  # (2, 4096, 8, 64)

    nc = _get_nc()
    ident = np.eye(P, dtype=np.float32)
    in_maps = []
    for c in range(8):
        b, hq = divmod(c, 4)
        h0 = hq * NH
        in_maps.append(
            {
                "q": np.ascontiguousarray(q[b, :, h0 : h0 + NH, :]),
                "k": np.ascontiguousarray(k[b, :, h0 : h0 + NH, :]),
                "v": np.ascontiguousarray(v[b, :, h0 : h0 + NH, :]),
                "iden": ident,
            }
        )
    res = run_bass_kernel_spmd(nc, in_maps, list(range(8)))
    out = np.empty((B, Lq, H, _E), np.float32)
    for c in range(8):
        b, hq = divmod(c, 4)
        h0 = hq * NH
        # core output is [NH, E+1, L]: rows 0..63 = unnormalized outT,
        # row 64 = softmax denominator. Normalize + transpose on host.
        ot = res.results[c]["o"]
        out[b, :, h0 : h0 + NH, :] = np.transpose(
            ot[:, :E, :] / ot[:, E : E + 1, :], (2, 0, 1)
        )
    return out
